# revision 1
# baseline (speedup 1.0000x reference)
"""Trainium2 Bass kernel for nn_Attn_head (GAT attention head, B=1) — v11 (v9 + double-buffered gather m1).

Same math as v2 (see kernel_v2.py docstring). v3 structural changes:
  - ONE 4KB D AllGather (the CC stream first-call cost dominates; per-half
    pipelining bought nothing).
  - Preamble PSUM tiles hold 4 j-blocks; one ACT copy per 4 blocks.
  - All Ga/Gb scales written by two broadcast-AP tensor_tensor ops into a
    single gall tile (lhsT slices for phase C), instead of 128 tiny ops.
  - SGb via a strided DVE reduce over gall + one tile-positioned matmul
    (replaces a 64-matmul PE chain).
"""

import sys
import numpy as np

for _p in ("/opt/trn_rl_repo", "/root/.axon_site/_ro/trn_rl_repo"):
    if _p not in sys.path:
        sys.path.insert(0, _p)

import concourse.bacc as bacc
import concourse.bass as bass
import concourse.mybir as mybir
import concourse.tile as tile
import concourse.masks as masks
import ml_dtypes
from concourse.bass_utils import run_bass_kernel_spmd

FP32 = mybir.dt.float32
BF16 = mybir.dt.bfloat16
ALU = mybir.AluOpType
AF = mybir.ActivationFunctionType

CIN = 128
COUT = 64
W = COUT + 1  # sft width per j-block: seq_ftsT cols + (-f) col
JBW = 128     # j-block width (PE contraction tile)
MF = 512      # moving free dim per matmul (one PSUM bank of fp32)
XCH = 1024    # x staging chunk (columns per DMA)
PBB = 4       # preamble j-blocks per PSUM tile
NE = 256      # suffix-sum edge grid size
NTB = NE // 128


def build(N=8192, CORES=8):
    nc = bacc.Bacc("TRN2", target_bir_lowering=False, debug=False,
                   num_devices=CORES)
    IC = N // CORES
    x_d = nc.dram_tensor("x", [CIN, N], BF16, kind="ExternalInput")
    xI_d = nc.dram_tensor("xI", [CIN, IC], BF16, kind="ExternalInput")
    w1_d = nc.dram_tensor("w1", [COUT, CIN], FP32, kind="ExternalInput")
    w1T_d = nc.dram_tensor("w1T", [CIN, COUT], FP32, kind="ExternalInput")
    w2T_d = nc.dram_tensor("w2T", [COUT, 1], FP32, kind="ExternalInput")
    ep_d = nc.dram_tensor("ep", [1, 4], FP32, kind="ExternalInput")
    y_d = nc.dram_tensor("y", [COUT, IC], FP32, kind="ExternalOutput")

    with tile.TileContext(nc) as tc:
        _build_body(tc, nc, x_d, xI_d, w1_d, w1T_d, w2T_d, ep_d, y_d, N, CORES)
    nc.compile()
    return nc


def _build_body(tc, nc, x_d, xI_d, w1_d, w1T_d, w2T_d, ep_d, y_d, N, CORES):
    from contextlib import ExitStack
    IC = N // CORES
    NJB = N // JBW
    NH = max(IC // MF, 1)   # phase C column halves
    MFi = min(MF, IC)

    ctx = ExitStack()
    with ctx:
        sb = ctx.enter_context(tc.tile_pool(name="sb", bufs=1))
        mpool = ctx.enter_context(tc.tile_pool(name="mpool", bufs=1))
        xpool = ctx.enter_context(tc.tile_pool(name="xpool", bufs=2))
        eppool = ctx.enter_context(tc.tile_pool(name="eppool", bufs=1))
        m1pool = ctx.enter_context(tc.tile_pool(name="m1pool", bufs=2))
        mbpool = ctx.enter_context(tc.tile_pool(name="mbpool", bufs=2))
        pre_ps_pool = ctx.enter_context(
            tc.tile_pool(name="pre_ps", bufs=1, space="PSUM"))
        fu_ps_pool = ctx.enter_context(
            tc.tile_pool(name="fu_ps", bufs=1, space="PSUM"))
        big_ps_pool = ctx.enter_context(
            tc.tile_pool(name="big_ps", bufs=1, space="PSUM"))
        misc_ps_pool = ctx.enter_context(
            tc.tile_pool(name="misc_ps", bufs=1, space="PSUM"))
        mb2_ps_pool = ctx.enter_context(
            tc.tile_pool(name="mb2_ps", bufs=1, space="PSUM"))
        qq_ps_pool = ctx.enter_context(
            tc.tile_pool(name="qq_ps", bufs=1, space="PSUM"))
        dram = ctx.enter_context(tc.tile_pool(name="dram", bufs=1, space="DRAM"))

        # ---------------- phase 0: weights ----------------
        w1_oc = sb.tile([COUT, CIN], FP32)
        nc.sync.dma_start(w1_oc[:, :], w1_d.ap())
        w1T_f = sb.tile([CIN, COUT], FP32)
        nc.sync.dma_start(w1T_f[:, :], w1T_d.ap())
        w2T = sb.tile([COUT, 1], FP32)
        nc.sync.dma_start(w2T[:, :], w2T_d.ap())

        wf_ps = misc_ps_pool.tile([CIN, 1], FP32, name="wf_ps", tag="m1")
        nc.tensor.matmul(wf_ps[:, :], w1_oc[:, :], w2T[:, :])
        wf_col = sb.tile([CIN, 1], FP32)
        nc.scalar.activation(wf_col[:, :], wf_ps[:, :], AF.Copy)
        wfull = sb.tile([CIN, W], BF16)
        nc.vector.tensor_copy(wfull[:, 0:COUT], w1T_f[:, :])
        nc.scalar.activation(wfull[:, COUT:W], wf_ps[:, :], AF.Copy, scale=-1.0)

        ones = sb.tile([128, 128], FP32)
        nc.gpsimd.memset(ones[:, :], 1.0)
        ones_bf = sb.tile([128, 1], BF16)
        nc.gpsimd.memset(ones_bf[:, :], 1.0)
        wf_rep = sb.tile([CIN, 128], BF16)
        nc.vector.tensor_scalar(wf_rep[:, :], ones[:, :], wf_col[:, 0:1], None,
                                ALU.mult)
        id64 = sb.tile([COUT, COUT], FP32)
        masks.make_identity(nc, id64[:, :])
        ep_sb = sb.tile([1, 4], FP32)
        nc.sync.dma_start(ep_sb[:, :], ep_d.ap())
        onesb = sb.tile([1, 128], BF16)
        nc.gpsimd.memset(onesb[:, :], 1.0)
        wf_colb = sb.tile([CIN, 1], BF16)
        nc.vector.tensor_copy(wf_colb[:, :], wf_col[:, :])
        epb_ps = mb2_ps_pool.tile([128, 4], FP32, name="epb_ps", tag="m2")
        nc.tensor.matmul(epb_ps[:, :], ones[0:1, :], ep_sb[:, :])
        epb = sb.tile([128, 4], FP32)
        nc.scalar.activation(epb[:, :], epb_ps[:, :], AF.Copy)
        it_e = xpool.tile([128, NE], mybir.dt.int32, name="it_e", tag="xb")
        nc.gpsimd.iota(it_e[:, :], pattern=[[1, NE]], channel_multiplier=0)
        E_bc = sb.tile([128, NE], BF16)   # E_bc[p,t] = e_t
        nc.scalar.activation(E_bc[:, :], it_e[:, :], AF.Identity,
                             bias=epb[:, 0:1], scale=epb[:, 1:2])
        it_p = sb.tile([128, NTB], mybir.dt.int32)
        nc.gpsimd.iota(it_p[:, :], pattern=[[128, NTB]], channel_multiplier=1)
        ecol = sb.tile([128, NTB], FP32)  # ecol[p,tb] = -e_(128*tb+p)
        nc.scalar.activation(ecol[:, :], it_p[:, :], AF.Identity,
                             bias=epb[:, 2:3], scale=epb[:, 3:4])
        id3 = sb.tile([3, 3], BF16)
        masks.make_identity(nc, id3[:, :])
        id2 = sb.tile([128, COUT], BF16)   # stacked double identity
        masks.make_identity(nc, id2[0:COUT, :])
        masks.make_identity(nc, id2[COUT:128, :])

        # ---------------- F broadcast (own i-shard) ----------------
        xI_sb = xpool.tile([CIN, IC], BF16, name="xI", tag="xs")
        nc.sync.dma_start(xI_sb[:, :], xI_d.ap())
        F_ps = fu_ps_pool.tile([128, IC], FP32, name="F_ps", tag="fu")
        for h in range(NH):
            sl = slice(h * MFi, (h + 1) * MFi)
            nc.tensor.matmul(F_ps[:, sl], wf_rep[:, :], xI_sb[:, sl])
        F_sb = sb.tile([128, IC], BF16)   # f[i] bcast over partitions
        nc.scalar.activation(F_sb[:, :], F_ps[:, :], AF.Copy)
        ab_bc = sb.tile([128, IC], FP32)  # rows 0:64 exp(f), 64:128 exp(.01f)
        nc.scalar.activation(ab_bc[0:COUT, :], F_ps[0:COUT, :], AF.Exp)
        nc.scalar.activation(ab_bc[COUT:128, :], F_ps[COUT:128, :], AF.Exp,
                             scale=0.01)
        ar = sb.tile([1, IC], BF16)
        nc.scalar.activation(ar[:, :], F_ps[0:1, :], AF.Exp)
        br = sb.tile([1, IC], BF16)
        nc.scalar.activation(br[:, :], F_ps[0:1, :], AF.Exp, scale=0.01)

        # ---------------- preamble + masks ----------------
        sft = sb.tile([128, NJB * W], BF16)   # [j_in_block, (JB, o|-f)]
        nf = sb.tile([128, NJB], FP32)        # -f[j] fp32 (mask scalars)
        a_all = sb.tile([128, NJB], FP32)
        b_all = sb.tile([128, NJB], FP32)
        pf = sb.tile([128, NJB], FP32)        # +f[j]
        abp = sb.tile([128, 2 * NJB], BF16)  # per jb: [a, b]
        F_ALL = sb.tile([128, N], BF16)      # f[j] bcast over partitions, all N
        U_e_ps = misc_ps_pool.tile([2, NE], FP32, name="U_e", tag="m1")
        m_tiles = []
        CPX = XCH // JBW
        for ch in range(NJB // CPX):
            j0 = ch * CPX
            xs = xpool.tile([CIN, XCH], BF16, name=f"xs{ch}", tag="xs")
            nc.sync.dma_start(xs[:, :], x_d.ap()[:, j0 * JBW:j0 * JBW + XCH])
            for g in range(CPX // PBB):
                jg = j0 + g * PBB
                pre_ps = pre_ps_pool.tile([128, PBB * W], FP32,
                                          name=f"pre{jg}", tag="pre")
                for k in range(PBB):
                    xo = (jg - j0 + k) * JBW
                    nc.tensor.matmul(pre_ps[:, k * W:(k + 1) * W],
                                     xs[:, xo:xo + JBW], wfull[:, :])
                nc.scalar.activation(
                    sft[:, jg * W:(jg + PBB) * W], pre_ps[:, :], AF.Copy)
            nc.scalar.activation(
                nf[:, j0:j0 + CPX],
                sft[:, j0 * W + COUT:(j0 + CPX) * W:W], AF.Copy)
            csl = slice(j0, j0 + CPX)
            nc.scalar.activation(a_all[:, csl], nf[:, csl], AF.Exp, scale=-1.0)
            nc.scalar.activation(b_all[:, csl], nf[:, csl], AF.Exp, scale=-0.01)
            nc.gpsimd.tensor_copy(abp[:, 2 * j0 + 0:2 * (j0 + CPX):2],
                                  a_all[:, csl])
            nc.gpsimd.tensor_copy(abp[:, 2 * j0 + 1:2 * (j0 + CPX):2],
                                  b_all[:, csl])
            nc.gpsimd.tensor_scalar(pf[:, csl], nf[:, csl], -1.0, None,
                                    ALU.mult)
            # F_ALL chunk: f o-major then partition-broadcast
            for g2 in range(XCH // MF):
                fom_ps = qq_ps_pool.tile([1, MF], FP32, name=f"fom{ch}{g2}",
                                         tag="qq")
                nc.tensor.matmul(fom_ps[:, :], wf_colb[:, :],
                                 xs[:, g2 * MF:(g2 + 1) * MF])
                frow = xpool.tile([1, MF], BF16, name=f"fr{ch}{g2}", tag="xb")
                nc.scalar.activation(frow[:, :], fom_ps[:, :], AF.Copy)
                fb_ps = qq_ps_pool.tile([128, MF], FP32, name=f"fb{ch}{g2}",
                                        tag="qq")
                nc.tensor.matmul(fb_ps[:, :], onesb[:, :], frow[:, :])
                nc.scalar.activation(
                    F_ALL[:, j0 * JBW + g2 * MF:j0 * JBW + (g2 + 1) * MF],
                    fb_ps[:, :], AF.Copy)
            for jb in range(j0, j0 + CPX):
                m = mpool.tile([128, IC], BF16, name=f"m{jb}", tag=f"m{jb}")
                nc.vector.tensor_scalar(
                    m[:, :], F_sb[:, :], nf[:, jb:jb + 1], None, ALU.is_ge)
                m_tiles.append(m)
                mb = mbpool.tile([128, NE], BF16, name=f"mb{jb}", tag="mb")
                nc.vector.tensor_scalar(
                    mb[:, :], E_bc[:, :], pf[:, jb:jb + 1], None, ALU.is_le)
                nc.tensor.matmul(U_e_ps[:, :], abp[:, 2 * jb:2 * jb + 2],
                                 mb[:, :],
                                 start=(jb == 0), stop=(jb == NJB - 1))

        # ---------------- Sb + suffix-sum D (no collective) ----------------
        b_red = sb.tile([128, 1], FP32)
        nc.vector.tensor_reduce(b_red[:, :], b_all[:, :], mybir.AxisListType.X,
                                ALU.add)
        Sb_ps = mb2_ps_pool.tile([1, 1], FP32, name="Sb_ps", tag="m2")
        nc.tensor.matmul(Sb_ps[:, :], b_red[:, :], ones[:, 0:1])
        Sb_sb = sb.tile([1, 1], FP32)
        nc.scalar.activation(Sb_sb[:, :], Sb_ps[:, :], AF.Copy)
        Sbb_ps = mb2_ps_pool.tile([128, 1], FP32, name="Sbb_ps", tag="m2")
        nc.tensor.matmul(Sbb_ps[:, :], ones[0:1, :], Sb_sb[:, :])
        Sb_bc = sb.tile([128, 1], FP32)
        nc.scalar.activation(Sb_bc[:, :], Sbb_ps[:, :], AF.Copy)

        # dU: Abel deltas with dU[0] = Ue[1] so gather = U(tau) directly
        Ue = sb.tile([2, NE], FP32)
        nc.scalar.activation(Ue[:, :], U_e_ps[:, :], AF.Copy)
        dU = sb.tile([2, NE], BF16)
        nc.vector.tensor_tensor(dU[:, 0:NE - 1], Ue[:, 1:NE], Ue[:, 0:NE - 1],
                                ALU.subtract)
        nc.vector.tensor_scalar(dU[:, NE - 1:NE], Ue[:, NE - 1:NE], -1.0,
                                None, ALU.mult)
        nc.vector.tensor_scalar(dU[:, 0:1], Ue[:, 1:2], 1.0, None, ALU.mult)
        dUT_ps = mb2_ps_pool.tile([128, 2 * NTB], BF16, name="dUT_ps",
                                  tag="m2")
        for tb in range(NTB):
            nc.tensor.transpose(dUT_ps[:, 2 * tb:2 * tb + 2],
                                dU[:, tb * 128:(tb + 1) * 128], id3[0:2, 0:2])
        dUT = sb.tile([128, 2 * NTB], BF16)
        nc.scalar.activation(dUT[:, :], dUT_ps[:, :], AF.Copy)

        # gather U(-f_j) for all j: m1[t, j] = [f_j <= -e_t], out [j-part, 2]
        U3_ps = misc_ps_pool.tile([128, 2 * NJB], FP32, name="U3", tag="m1")
        HCW = CPX * JBW // 2   # m1 half-chunk width
        HCB = HCW // JBW       # j-blocks per half-chunk
        for hc in range(N // HCW):
            cb = hc * HCW
            m1s = []
            for tb in range(NTB):
                m1t = m1pool.tile([128, HCW], BF16,
                                  name=f"m1_{hc}_{tb}", tag=f"m1{tb}")
                nc.vector.tensor_scalar(
                    m1t[:, :], F_ALL[:, cb:cb + HCW],
                    ecol[:, tb:tb + 1], None, ALU.is_le)
                m1s.append(m1t)
            for jb2 in range(HCB):
                jb = hc * HCB + jb2
                for tb in range(NTB):
                    nc.tensor.matmul(
                        U3_ps[:, 2 * jb:2 * jb + 2],
                        m1s[tb][:, jb2 * JBW:(jb2 + 1) * JBW],
                        dUT[:, 2 * tb:2 * tb + 2],
                        start=(tb == 0), stop=(tb == NTB - 1))
        U3 = sb.tile([128, 2 * NJB], FP32)
        nc.scalar.activation(U3[:, :], U3_ps[:, :], AF.Copy)
        # D_T[j] = a_j*SA(tau_j) + b_j*(Sb - SB(tau_j))
        t1 = sb.tile([128, NJB], FP32)
        nc.vector.tensor_tensor(t1[:, :], a_all[:, :], U3[:, 0::2], ALU.mult)
        s2 = sb.tile([128, NJB], FP32)
        nc.vector.tensor_scalar(s2[:, :], U3[:, 1::2], Sb_bc[:, 0:1], -1.0,
                                ALU.subtract, ALU.mult)
        t2 = sb.tile([128, NJB], FP32)
        nc.vector.tensor_tensor(t2[:, :], b_all[:, :], s2[:, :], ALU.mult)
        D_T = sb.tile([128, NJB], FP32)
        nc.vector.tensor_tensor(D_T[:, :], t1[:, :], t2[:, :], ALU.add)
        Dinv = sb.tile([128, NJB], FP32)
        nc.vector.reciprocal(Dinv[:, :], D_T[:, :])
        aDb = sb.tile([128, NJB], FP32)
        nc.vector.tensor_tensor(aDb[:, :], a_all[:, :], Dinv[:, :], ALU.mult)
        bDb = sb.tile([128, NJB], FP32)
        nc.vector.tensor_tensor(bDb[:, :], b_all[:, :], Dinv[:, :], ALU.mult)

        # ---------------- gall chunks: [Ga | Gb] per jb ----------------
        GCH = CPX  # j-blocks per gall chunk
        NGC = NJB // GCH
        gtiles = []
        for c in range(NGC):
            j0 = c * GCH
            gc = sb.tile([128, GCH * 2 * COUT], BF16, name=f"gall{c}")
            gvv = gc[:, :].rearrange("p (j t) -> p j t", t=2 * COUT)
            sfv = sft[:, j0 * W:(j0 + GCH) * W].rearrange(
                "p (j w) -> p j w", w=W)[:, :, 0:COUT]
            nc.vector.tensor_tensor(
                gvv[:, :, 0:COUT], sfv,
                aDb[:, j0:j0 + GCH].unsqueeze(2).broadcast_to(
                    [128, GCH, COUT]), ALU.mult)
            nc.vector.tensor_tensor(
                gvv[:, :, COUT:2 * COUT], sfv,
                bDb[:, j0:j0 + GCH].unsqueeze(2).broadcast_to(
                    [128, GCH, COUT]), ALU.mult)
            gtiles.append(gc)

        # SGb[o] = sum_j Gb[j, o]: per-chunk strided reduces + final combine
        sgp = sb.tile([128, NGC * COUT], FP32)
        for c in range(NGC):
            nc.vector.tensor_reduce(
                sgp[:, c * COUT:(c + 1) * COUT],
                gtiles[c][:, :].rearrange(
                    "p (j t) -> p t j", t=2 * COUT)[:, COUT:, :],
                mybir.AxisListType.X, ALU.add)
        sgr = sb.tile([128, COUT], FP32)
        nc.vector.tensor_reduce(
            sgr[:, :],
            sgp[:, :].rearrange("p (c o) -> p o c", o=COUT),
            mybir.AxisListType.X, ALU.add)
        sg_ps = misc_ps_pool.tile([128, 1], FP32, name="sg_ps", tag="m1")
        nc.tensor.matmul(sg_ps[COUT:128, 0:1], sgr[:, :], ones[:, 0:1],
                         tile_position=(0, 64))
        sgb_col = sb.tile([128, 1], FP32)
        nc.scalar.activation(sgb_col[COUT:128, :], sg_ps[COUT:128, :], AF.Copy)

        # ---------------- phase C + epilogue per column half ----------------
        out_ps = big_ps_pool.tile([128, IC], FP32, name="out_ps", tag="big")
        for h2 in range(NH):
            sl2 = slice(h2 * MFi, (h2 + 1) * MFi)
            for jb in range(NJB):
                gt = gtiles[jb // GCH]
                go = (jb % GCH) * 2 * COUT
                nc.tensor.matmul(out_ps[:, sl2],
                                 gt[:, go:go + 2 * COUT],
                                 m_tiles[jb][:, sl2],
                                 start=(jb == 0), stop=(jb == NJB - 1))
            tfu = eppool.tile([128, MFi], BF16, name=f"tf{h2}", tag="e1")
            nc.vector.tensor_tensor(tfu[0:COUT, :], ab_bc[0:COUT, sl2],
                                    out_ps[0:COUT, sl2], ALU.mult)
            eb = eppool.tile([128, MFi], FP32, name=f"eb{h2}", tag="e2")
            nc.scalar.activation(eb[COUT:128, :], out_ps[COUT:128, sl2],
                                 AF.Identity, bias=sgb_col[COUT:128, 0:1],
                                 scale=-1.0)
            nc.vector.tensor_tensor(tfu[COUT:128, :], ab_bc[COUT:128, sl2],
                                    eb[COUT:128, :], ALU.mult)
            z_ps = qq_ps_pool.tile([COUT, MFi], FP32, name=f"z{h2}", tag="qq")
            nc.tensor.matmul(z_ps[:, :], id2[:, :], tfu[:, :])
            e = eppool.tile([COUT, MFi], BF16, name=f"e{h2}", tag="e3")
            nc.scalar.activation(e[:, :], z_ps[:, :], AF.Exp)
            r = eppool.tile([COUT, MFi], BF16, name=f"r{h2}", tag="e4")
            nc.scalar.activation(r[:, :], z_ps[:, :], AF.Relu)
            q = eppool.tile([COUT, MFi], BF16, name=f"q{h2}", tag="e5")
            nc.vector.tensor_scalar(q[:, :], e[:, :], 1.0, -1.0, ALU.min,
                                    ALU.add)
            y_sb = eppool.tile([COUT, MFi], FP32, name=f"y{h2}", tag="e6")
            nc.vector.tensor_tensor(y_sb[:, :], r[:, :], q[:, :], ALU.add)
            nc.sync.dma_start(y_d.ap()[:, sl2], y_sb[:, :])


_NC_CACHE = {}


def _get_nc(N, CORES):
    key = (N, CORES)
    if key not in _NC_CACHE:
        _NC_CACHE[key] = build(N, CORES)
    return _NC_CACHE[key]


def _numpy_fallback(x, bias_mat, w1, w2_1):
    x2 = x[0].astype(np.float64)
    seq = w1.astype(np.float64) @ x2
    f = (w2_1.astype(np.float64) @ seq)[0]
    logits = f[:, None] + f[None, :]
    lr = np.where(logits >= 0, logits, 0.01 * logits) + bias_mat.astype(np.float64)
    e = np.exp(lr - lr.max(axis=0, keepdims=True))
    coefs = e / e.sum(axis=0, keepdims=True)
    ret = np.einsum('ij,oj->oi', coefs, seq)
    out = np.where(ret > 0, ret, np.exp(np.minimum(ret, 0)) - 1)
    return out[None].astype(np.float32)


def kernel(x, bias_mat, w1, w2_1, **_ignored):
    x = np.ascontiguousarray(np.asarray(x, dtype=np.float32))
    w1 = np.ascontiguousarray(np.asarray(w1, dtype=np.float32))
    w2_1 = np.ascontiguousarray(np.asarray(w2_1, dtype=np.float32))
    bias_mat = np.asarray(bias_mat)
    if bias_mat.size and np.any(bias_mat):
        return _numpy_fallback(x, bias_mat, w1, w2_1)
    B, cin, N = x.shape
    assert B == 1 and cin == CIN
    CORES = 8
    IC = N // CORES
    x2 = x[0]

    nc = _get_nc(N, CORES)
    xbf = x2.astype(ml_dtypes.bfloat16)
    wf = (w2_1 @ w1)[0]
    f = wf @ x2
    fmax = float(np.abs(f).max()) * 1.05 + 0.05
    ep = np.array([[-fmax, 2.0 * fmax / NE, fmax, -2.0 * fmax / NE]],
                  dtype=np.float32)
    in_maps = []
    for c in range(CORES):
        in_maps.append({
            "x": xbf,
            "xI": np.ascontiguousarray(xbf[:, c * IC:(c + 1) * IC]),
            "w1": w1,
            "w1T": np.ascontiguousarray(w1.T),
            "w2T": np.ascontiguousarray(w2_1.T),
            "ep": ep,
        })
    res = run_bass_kernel_spmd(nc, in_maps, core_ids=list(range(CORES)))
    y = np.concatenate([res.results[c]["y"] for c in range(CORES)], axis=1)
    return y[None].astype(np.float32)


if __name__ == "__main__":
    rng = np.random.default_rng(0)
    N = 8192
    x = rng.standard_normal((1, CIN, N), dtype=np.float32)
    w1 = (rng.standard_normal((COUT, CIN)) / np.sqrt(CIN)).astype(np.float32)
    w2 = (rng.standard_normal((1, COUT)) / np.sqrt(COUT)).astype(np.float32)
    bias = np.zeros((N, N), np.float32)
    y = kernel(x=x, bias_mat=bias, w1=w1, w2_1=w2)
    print("kernel output", y.shape, y.dtype)



# revision 12
# speedup vs baseline: 1.7354x; 1.7354x over previous
"""Trainium2 Bass kernel for nn_Attn_head (GAT attention head, B=1) — v12.

v11 -> v12: quantized table-gather for the numerator (phase C).
  - TG[c,t] = sum_j G[j,c]*[f_j >= e_t] accumulated by 64 matmuls over
    the SAME mb masks already built for U_e (NE=128 grid, kept
    resident), instead of 64 exact [128,1024] DVE masks + a 64-deep
    512-col matmul chain per half.
  - out[c,i] = TG[c, t*(i)+1] via Abel deltas (dTG) + one mt2 mask and
    two 512-col matmuls.
  - F_ALL broadcast by ONE stationary wf_rep matmul per 512 cols
    (replaces the frow/fb two-matmul + two-ACT chain).
  - SGb/Sb read from sentinel (always-true) mask columns of the table
    matmuls (replaces tensor_reduce + matmul reduction chains).
  - NE 256 -> 128 (error budget dominated by bf16, not the grid).
  - dead ar/br removed.
"""

import sys
import numpy as np

for _p in ("/opt/trn_rl_repo", "/root/.axon_site/_ro/trn_rl_repo"):
    if _p not in sys.path:
        sys.path.insert(0, _p)

import concourse.bacc as bacc
import concourse.bass as bass
import concourse.mybir as mybir
import concourse.tile as tile
import concourse.masks as masks
import ml_dtypes
from concourse.bass_utils import run_bass_kernel_spmd

FP32 = mybir.dt.float32
BF16 = mybir.dt.bfloat16
INT32 = mybir.dt.int32
ALU = mybir.AluOpType
AF = mybir.ActivationFunctionType

CIN = 128
COUT = 64
W = COUT + 1  # sft width per j-block: seq_ftsT cols + (-f) col
JBW = 128     # j-block width (PE contraction tile)
MF = 512      # moving free dim per matmul (one PSUM bank of fp32)
XCH = 1024    # x staging chunk (columns per DMA)
PBB = 4       # preamble j-blocks per PSUM tile
NE = 128      # grid size (one partition tile)
NEX = NE + 8  # +8 sentinel always-true cols (col NE => column sums)
DEBUG_DUMP = False
_DBG = {}


def build(N=8192, CORES=8):
    nc = bacc.Bacc("TRN2", target_bir_lowering=False, debug=False,
                   num_devices=CORES)
    IC = N // CORES
    x_d = nc.dram_tensor("x", [CIN, N], BF16, kind="ExternalInput")
    xI_d = nc.dram_tensor("xI", [CIN, IC], BF16, kind="ExternalInput")
    w1_d = nc.dram_tensor("w1", [COUT, CIN], FP32, kind="ExternalInput")
    w1T_d = nc.dram_tensor("w1T", [CIN, COUT], FP32, kind="ExternalInput")
    w2T_d = nc.dram_tensor("w2T", [COUT, 1], FP32, kind="ExternalInput")
    ep_d = nc.dram_tensor("ep", [1, 4], FP32, kind="ExternalInput")
    y_d = nc.dram_tensor("y", [COUT, IC], FP32, kind="ExternalOutput")
    if DEBUG_DUMP:
        _DBG.clear()
        for nm, shp in [("dUe", [2, NEX]), ("dU3", [128, 128]),
                        ("dTGd", [128, NEX]), ("dnf", [128, 64]),
                        ("dDT", [128, 64]), ("dFA", [128, 1024]),
                        ("dM1", [128, 1024]), ("dOP", [128, 512]),
                        ("dEB", [128, NEX]), ("dMB", [128, NEX]),
                        ("dABP", [128, 128]), ("dPF", [128, 64])]:
            _DBG[nm] = nc.dram_tensor(nm, shp, FP32, kind="ExternalOutput")

    with tile.TileContext(nc) as tc:
        _build_body(tc, nc, x_d, xI_d, w1_d, w1T_d, w2T_d, ep_d, y_d, N, CORES)
    nc.compile()
    return nc


def _build_body(tc, nc, x_d, xI_d, w1_d, w1T_d, w2T_d, ep_d, y_d, N, CORES):
    from contextlib import ExitStack
    IC = N // CORES
    NJB = N // JBW
    NCH = N // XCH
    CPX = XCH // JBW
    NH = max(IC // MF, 1)
    MFi = min(MF, IC)

    ctx = ExitStack()
    with ctx:
        sb = ctx.enter_context(tc.tile_pool(name="sb", bufs=1))
        xpool = ctx.enter_context(tc.tile_pool(name="xpool", bufs=3))
        mbpool = ctx.enter_context(tc.tile_pool(name="mbpool", bufs=1))
        m1pool = ctx.enter_context(tc.tile_pool(name="m1pool", bufs=1))
        eppool = ctx.enter_context(tc.tile_pool(name="eppool", bufs=2))
        ue_ps_pool = ctx.enter_context(
            tc.tile_pool(name="ue_ps", bufs=1, space="PSUM"))
        pre_ps_pool = ctx.enter_context(
            tc.tile_pool(name="pre_ps", bufs=2, space="PSUM"))
        fa_ps_pool = ctx.enter_context(
            tc.tile_pool(name="fa_ps", bufs=2, space="PSUM"))
        big_ps_pool = ctx.enter_context(
            tc.tile_pool(name="big_ps", bufs=1, space="PSUM"))
        misc_ps_pool = ctx.enter_context(
            tc.tile_pool(name="misc_ps", bufs=1, space="PSUM"))

        # ---------------- phase 0: weights & grid ----------------
        w1_oc = sb.tile([COUT, CIN], FP32)
        nc.sync.dma_start(w1_oc[:, :], w1_d.ap())
        w1T_f = sb.tile([CIN, COUT], FP32)
        nc.sync.dma_start(w1T_f[:, :], w1T_d.ap())
        w2T = sb.tile([COUT, 1], FP32)
        nc.sync.dma_start(w2T[:, :], w2T_d.ap())
        ep_sb = sb.tile([1, 4], FP32)
        nc.sync.dma_start(ep_sb[:, :], ep_d.ap())
        xI_sb = xpool.tile([CIN, IC], BF16, name="xI", tag="xi")
        nc.sync.dma_start(xI_sb[:, :], xI_d.ap())

        ones = sb.tile([128, 128], FP32)
        nc.gpsimd.memset(ones[:, :], 1.0)
        idT = sb.tile([128, 128], BF16)
        masks.make_identity(nc, idT[:, :])
        id2 = sb.tile([128, COUT], BF16)   # stacked double identity
        masks.make_identity(nc, id2[0:COUT, :])
        masks.make_identity(nc, id2[COUT:128, :])
        it_e = sb.tile([128, NE], INT32)
        nc.gpsimd.iota(it_e[:, :], pattern=[[1, NE]], channel_multiplier=0)
        it_p = sb.tile([128, 1], INT32)
        nc.gpsimd.iota(it_p[:, :], pattern=[[128, 1]], channel_multiplier=1)

        wf_ps = misc_ps_pool.tile([CIN, 1], FP32, name="wf_ps", tag="mm")
        nc.tensor.matmul(wf_ps[:, :], w1_oc[:, :], w2T[:, :])
        wf_col = sb.tile([CIN, 1], FP32)
        nc.scalar.activation(wf_col[:, :], wf_ps[:, :], AF.Copy)
        wfull = sb.tile([CIN, W], BF16)
        nc.vector.tensor_copy(wfull[:, 0:COUT], w1T_f[:, :])
        nc.scalar.activation(wfull[:, COUT:W], wf_ps[:, :], AF.Copy, scale=-1.0)
        wf_rep = sb.tile([CIN, 128], BF16)
        nc.vector.tensor_scalar(wf_rep[:, :], ones[:, :], wf_col[:, 0:1], None,
                                ALU.mult)

        epb_ps = misc_ps_pool.tile([128, 4], FP32, name="epb_ps", tag="mm")
        nc.tensor.matmul(epb_ps[:, :], ones[0:1, :], ep_sb[:, :])
        epb = sb.tile([128, 4], FP32)
        nc.scalar.activation(epb[:, :], epb_ps[:, :], AF.Copy)
        E_bc = sb.tile([128, NEX], BF16)   # E_bc[p,t] = e_t; cols NE.. = -inf
        nc.scalar.activation(E_bc[:, 0:NE], it_e[:, :], AF.Identity,
                             bias=epb[:, 0:1], scale=epb[:, 1:2])
        nc.gpsimd.memset(E_bc[:, NE:NEX], -1.0e38)
        ecol = sb.tile([128, 1], FP32)     # ecol[p] = -e_p
        nc.scalar.activation(ecol[:, :], it_p[:, :], AF.Identity,
                             bias=epb[:, 2:3], scale=epb[:, 3:4])

        # ---------------- F broadcast (own i-shard) ----------------
        F_sb = sb.tile([128, IC], BF16)    # f[i] bcast over partitions
        ab_bc = sb.tile([128, IC], FP32)   # rows 0:64 exp(f), 64:128 exp(.01f)
        for h in range(NH):
            sl = slice(h * MFi, (h + 1) * MFi)
            fi_ps = fa_ps_pool.tile([128, MFi], FP32, name=f"fi{h}", tag="fa")
            nc.tensor.matmul(fi_ps[:, :], wf_rep[:, :], xI_sb[:, sl])
            nc.scalar.activation(F_sb[:, sl], fi_ps[:, :], AF.Copy)
            nc.scalar.activation(ab_bc[0:COUT, sl], fi_ps[0:COUT, :], AF.Exp)
            nc.scalar.activation(ab_bc[COUT:128, sl], fi_ps[COUT:128, :],
                                 AF.Exp, scale=0.01)
        mt2 = sb.tile([128, IC], BF16)     # mt2[p,i] = [e_p <= -f_i]
        nc.vector.tensor_scalar(mt2[:, :], F_sb[:, :], ecol[:, 0:1], None,
                                ALU.is_le)

        # ---------------- preamble chunk loop ----------------
        sft = sb.tile([128, NJB * W], BF16)   # [j_in_block, (JB, o|-f)]
        F_ALL = sb.tile([128, N], BF16)       # f[j] bcast over partitions
        nf = sb.tile([128, NJB], FP32)        # -f[j]
        a_all = sb.tile([128, NJB], FP32)
        b_all = sb.tile([128, NJB], FP32)
        pf = sb.tile([128, NJB], FP32)        # +f[j]
        abp = sb.tile([128, 2 * NJB], BF16)   # per jb: [b, a] lhsT cols
        Ue_ps = ue_ps_pool.tile([2, NEX], FP32, name="Ue_ps", tag="ue")
        mb_tiles = []
        m1_tiles = []
        for ch in range(NCH):
            j0 = ch * CPX
            xs = xpool.tile([CIN, XCH], BF16, name=f"xs{ch}", tag="xs")
            nc.sync.dma_start(xs[:, :], x_d.ap()[:, ch * XCH:(ch + 1) * XCH])
            # F_ALL chunk: stationary wf_rep, one matmul per 512 cols
            for g in range(XCH // MF):
                fa = fa_ps_pool.tile([128, MF], FP32, name=f"fa{ch}{g}",
                                     tag="fa")
                nc.tensor.matmul(fa[:, :], wf_rep[:, :],
                                 xs[:, g * MF:(g + 1) * MF])
                dst = F_ALL[:, ch * XCH + g * MF:ch * XCH + (g + 1) * MF]
                nc.scalar.activation(dst, fa[:, :], AF.Copy)
            # m1 chunk: m1[p, j] = [e_p <= -f_j]
            m1c = m1pool.tile([128, XCH], BF16, name=f"m1_{ch}", tag=f"m1{ch}")
            nc.vector.tensor_scalar(
                m1c[:, :], F_ALL[:, ch * XCH:(ch + 1) * XCH], ecol[:, 0:1],
                None, ALU.is_le)
            m1_tiles.append(m1c)
            # seq_ftsT + (-f) per PBB-group
            for g in range(CPX // PBB):
                jg = j0 + g * PBB
                pre = pre_ps_pool.tile([128, PBB * W], FP32, name=f"pre{jg}",
                                       tag="pre")
                for k in range(PBB):
                    xo = (g * PBB + k) * JBW
                    nc.tensor.matmul(pre[:, k * W:(k + 1) * W],
                                     xs[:, xo:xo + JBW], wfull[:, :])
                dst = sft[:, jg * W:(jg + PBB) * W]
                nc.scalar.activation(dst, pre[:, :], AF.Copy)
            csl = slice(j0, j0 + CPX)
            nc.scalar.activation(
                nf[:, csl], sft[:, j0 * W + COUT:(j0 + CPX) * W:W], AF.Copy)
            nc.scalar.activation(a_all[:, csl], nf[:, csl], AF.Exp, scale=-1.0)
            nc.scalar.activation(b_all[:, csl], nf[:, csl], AF.Exp,
                                 scale=-0.01)
            nc.gpsimd.tensor_scalar(pf[:, csl], nf[:, csl], -1.0, None,
                                    ALU.mult)
            nc.gpsimd.tensor_copy(abp[:, 2 * j0 + 0:2 * (j0 + CPX):2],
                                  b_all[:, csl])
            nc.gpsimd.tensor_copy(abp[:, 2 * j0 + 1:2 * (j0 + CPX):2],
                                  a_all[:, csl])
            for jb in range(j0, j0 + CPX):
                mb = mbpool.tile([128, NEX], BF16, name=f"mb{jb}",
                                 tag=f"mb{jb}")
                nc.vector.tensor_scalar(mb[:, :], E_bc[:, :],
                                        pf[:, jb:jb + 1], None, ALU.is_le)
                nc.tensor.matmul(Ue_ps[:, :], abp[:, 2 * jb:2 * jb + 2],
                                 mb[:, :],
                                 start=(jb == 0), stop=(jb == NJB - 1))
                mb_tiles.append(mb)

        # ---------------- U_e -> dU -> U3 -> D ----------------
        Ue = sb.tile([2, NEX], FP32)
        nc.scalar.activation(Ue[:, :], Ue_ps[:, :], AF.Copy)
        sb_ps = misc_ps_pool.tile([128, 1], FP32, name="sb_ps", tag="mm")
        nc.tensor.matmul(sb_ps[:, :], ones[0:1, :], Ue[0:1, 0:1])
        Sb_bc = sb.tile([128, 1], FP32)
        nc.scalar.activation(Sb_bc[:, :], sb_ps[:, :], AF.Copy)
        dU = sb.tile([2, NE], BF16)
        nc.vector.tensor_tensor(dU[:, 1:NE - 1], Ue[:, 2:NE], Ue[:, 1:NE - 1],
                                ALU.subtract)
        nc.vector.tensor_scalar(dU[:, NE - 1:NE], Ue[:, NE - 1:NE], -1.0,
                                None, ALU.mult)
        nc.vector.tensor_scalar(dU[:, 0:1], Ue[:, 1:2], 1.0, None, ALU.mult)
        dUT_ps = misc_ps_pool.tile([128, 2], BF16, name="dUT_ps", tag="mm")
        nc.tensor.transpose(dUT_ps[:, :], dU[:, :], idT[0:2, 0:2])
        dUT = sb.tile([128, 2], BF16)
        nc.scalar.activation(dUT[:, :], dUT_ps[:, :], AF.Copy)

        U3_ps = misc_ps_pool.tile([128, 2 * NJB], FP32, name="U3", tag="mm")
        for jb in range(NJB):
            nc.tensor.matmul(
                U3_ps[:, 2 * jb:2 * jb + 2],
                m1_tiles[jb // CPX][:, (jb % CPX) * JBW:(jb % CPX + 1) * JBW],
                dUT[:, :])
        U3 = sb.tile([128, 2 * NJB], FP32)
        nc.scalar.activation(U3[:, :], U3_ps[:, :], AF.Copy)
        if DEBUG_DUMP:
            deb = sb.tile([128, NEX], FP32)
            nc.vector.tensor_copy(deb[:, :], E_bc[:, :])
            nc.sync.dma_start(_DBG["dEB"].ap(), deb[:, :])
            dmb = sb.tile([128, NEX], FP32)
            nc.vector.tensor_copy(dmb[:, :], mb_tiles[0][:, :])
            nc.sync.dma_start(_DBG["dMB"].ap(), dmb[:, :])
            dabp = sb.tile([128, 128], FP32)
            nc.vector.tensor_copy(dabp[:, 0:2 * NJB], abp[:, 0:2 * NJB])
            nc.sync.dma_start(_DBG["dABP"].ap()[:, 0:2 * NJB], dabp[:, 0:2 * NJB])
            nc.sync.dma_start(_DBG["dPF"].ap()[:, 0:NJB], pf[:, :])
            nc.sync.dma_start(_DBG["dUe"].ap(), Ue[:, :])
            nc.sync.dma_start(_DBG["dU3"].ap()[:, 0:2 * NJB], U3[:, :])
            nc.sync.dma_start(_DBG["dnf"].ap()[:, 0:NJB], nf[:, :])
            dfa = sb.tile([128, 1024], FP32)
            nc.vector.tensor_copy(dfa[:, :], F_ALL[:, 0:1024])
            nc.sync.dma_start(_DBG["dFA"].ap(), dfa[:, :])
            dm1 = sb.tile([128, 1024], FP32)
            nc.vector.tensor_copy(dm1[:, :], m1_tiles[0][:, :])
            nc.sync.dma_start(_DBG["dM1"].ap(), dm1[:, :])
        # D_T[j] = a_j*UA(tau_j) + b_j*(Sb - UB(tau_j))
        t1 = sb.tile([128, NJB], FP32)
        nc.vector.tensor_tensor(t1[:, :], a_all[:, :], U3[:, 1::2], ALU.mult)
        s2 = sb.tile([128, NJB], FP32)
        nc.vector.tensor_scalar(s2[:, :], U3[:, 0::2], Sb_bc[:, 0:1], -1.0,
                                ALU.subtract, ALU.mult)
        t2 = sb.tile([128, NJB], FP32)
        nc.vector.tensor_tensor(t2[:, :], b_all[:, :], s2[:, :], ALU.mult)
        D_T = sb.tile([128, NJB], FP32)
        nc.vector.tensor_tensor(D_T[:, :], t1[:, :], t2[:, :], ALU.add)
        Dinv = sb.tile([128, NJB], FP32)
        nc.vector.reciprocal(Dinv[:, :], D_T[:, :])
        aDb = sb.tile([128, NJB], FP32)
        nc.vector.tensor_tensor(aDb[:, :], a_all[:, :], Dinv[:, :], ALU.mult)
        bDb = sb.tile([128, NJB], FP32)
        nc.vector.tensor_tensor(bDb[:, :], b_all[:, :], Dinv[:, :], ALU.mult)

        # ---------------- gall + TG table build ----------------
        TG_ps = misc_ps_pool.tile([128, NEX], FP32, name="TG_ps", tag="mm")
        for c in range(NCH):
            j0 = c * CPX
            gc = sb.tile([128, CPX * 2 * COUT], BF16, name=f"gall{c}")
            gvv = gc[:, :].rearrange("p (j t) -> p j t", t=2 * COUT)
            sfv = sft[:, j0 * W:(j0 + CPX) * W].rearrange(
                "p (j w) -> p j w", w=W)[:, :, 0:COUT]
            nc.vector.tensor_tensor(
                gvv[:, :, 0:COUT], sfv,
                aDb[:, j0:j0 + CPX].unsqueeze(2).broadcast_to(
                    [128, CPX, COUT]), ALU.mult)
            nc.vector.tensor_tensor(
                gvv[:, :, COUT:2 * COUT], sfv,
                bDb[:, j0:j0 + CPX].unsqueeze(2).broadcast_to(
                    [128, CPX, COUT]), ALU.mult)
            for jb in range(j0, j0 + CPX):
                go = (jb - j0) * 2 * COUT
                nc.tensor.matmul(TG_ps[:, :], gc[:, go:go + 2 * COUT],
                                 mb_tiles[jb][:, :],
                                 start=(jb == 0), stop=(jb == NJB - 1))
        sgb_col = sb.tile([128, 1], FP32)   # rows 64:128 = SGb
        nc.scalar.activation(sgb_col[COUT:128, :], TG_ps[COUT:128, NE:NE + 1],
                             AF.Copy)
        TGs = sb.tile([128, NEX], FP32)
        nc.scalar.activation(TGs[:, :], TG_ps[:, :], AF.Copy)
        if DEBUG_DUMP:
            nc.sync.dma_start(_DBG["dTGd"].ap(), TGs[:, :])
            nc.sync.dma_start(_DBG["dDT"].ap()[:, 0:NJB], D_T[:, :])
        dTG = sb.tile([128, NE], BF16)
        nc.vector.tensor_tensor(dTG[:, 1:NE - 1], TGs[:, 2:NE],
                                TGs[:, 1:NE - 1], ALU.subtract)
        nc.vector.tensor_scalar(dTG[:, NE - 1:NE], TGs[:, NE - 1:NE], -1.0,
                                None, ALU.mult)
        nc.vector.tensor_scalar(dTG[:, 0:1], TGs[:, 1:2], 1.0, None, ALU.mult)
        dTGT_ps = misc_ps_pool.tile([128, 128], BF16, name="dTGT_ps", tag="mm")
        nc.tensor.transpose(dTGT_ps[:, :], dTG[:, :], idT[:, :])
        dTGT = sb.tile([128, 128], BF16)
        nc.scalar.activation(dTGT[:, :], dTGT_ps[:, :], AF.Copy)

        # ---------------- gather + epilogue per column half ----------------
        out_ps = big_ps_pool.tile([128, IC], FP32, name="out_ps", tag="big")
        for h2 in range(NH):
            sl2 = slice(h2 * MFi, (h2 + 1) * MFi)
            nc.tensor.matmul(out_ps[:, sl2], dTGT[:, :], mt2[:, sl2])
            if DEBUG_DUMP and h2 == 0:
                dop = sb.tile([128, MFi], FP32)
                nc.vector.tensor_copy(dop[:, :], out_ps[:, 0:MFi])
                nc.sync.dma_start(_DBG["dOP"].ap()[:, 0:MFi], dop[:, :])
            tfu = eppool.tile([128, MFi], BF16, name=f"tf{h2}", tag="e1")
            nc.vector.tensor_tensor(tfu[0:COUT, :], ab_bc[0:COUT, sl2],
                                    out_ps[0:COUT, sl2], ALU.mult)
            eb = eppool.tile([128, MFi], FP32, name=f"eb{h2}", tag="e2")
            nc.scalar.activation(eb[COUT:128, :], out_ps[COUT:128, sl2],
                                 AF.Identity, bias=sgb_col[COUT:128, 0:1],
                                 scale=-1.0)
            nc.vector.tensor_tensor(tfu[COUT:128, :], ab_bc[COUT:128, sl2],
                                    eb[COUT:128, :], ALU.mult)
            z_ps = misc_ps_pool.tile([COUT, MFi], FP32, name=f"z{h2}",
                                     tag="mm")
            nc.tensor.matmul(z_ps[:, :], id2[:, :], tfu[:, :])
            e = eppool.tile([COUT, MFi], BF16, name=f"e{h2}", tag="e3")
            nc.scalar.activation(e[:, :], z_ps[:, :], AF.Exp)
            r = eppool.tile([COUT, MFi], BF16, name=f"r{h2}", tag="e4")
            nc.scalar.activation(r[:, :], z_ps[:, :], AF.Relu)
            q = eppool.tile([COUT, MFi], BF16, name=f"q{h2}", tag="e5")
            nc.vector.tensor_scalar(q[:, :], e[:, :], 1.0, -1.0, ALU.min,
                                    ALU.add)
            y_sb = eppool.tile([COUT, MFi], FP32, name=f"y{h2}", tag="e6")
            nc.vector.tensor_tensor(y_sb[:, :], r[:, :], q[:, :], ALU.add)
            nc.sync.dma_start(y_d.ap()[:, sl2], y_sb[:, :])


_NC_CACHE = {}


def _get_nc(N, CORES):
    key = (N, CORES)
    if key not in _NC_CACHE:
        _NC_CACHE[key] = build(N, CORES)
    return _NC_CACHE[key]


def _numpy_fallback(x, bias_mat, w1, w2_1):
    x2 = x[0].astype(np.float64)
    seq = w1.astype(np.float64) @ x2
    f = (w2_1.astype(np.float64) @ seq)[0]
    logits = f[:, None] + f[None, :]
    lr = np.where(logits >= 0, logits, 0.01 * logits) + bias_mat.astype(np.float64)
    e = np.exp(lr - lr.max(axis=0, keepdims=True))
    coefs = e / e.sum(axis=0, keepdims=True)
    ret = np.einsum('ij,oj->oi', coefs, seq)
    out = np.where(ret > 0, ret, np.exp(np.minimum(ret, 0)) - 1)
    return out[None].astype(np.float32)


def kernel(x, bias_mat, w1, w2_1, **_ignored):
    x = np.ascontiguousarray(np.asarray(x, dtype=np.float32))
    w1 = np.ascontiguousarray(np.asarray(w1, dtype=np.float32))
    w2_1 = np.ascontiguousarray(np.asarray(w2_1, dtype=np.float32))
    bias_mat = np.asarray(bias_mat)
    if bias_mat.size and np.any(bias_mat):
        return _numpy_fallback(x, bias_mat, w1, w2_1)
    B, cin, N = x.shape
    assert B == 1 and cin == CIN
    CORES = 8
    IC = N // CORES
    x2 = x[0]

    nc = _get_nc(N, CORES)
    xbf = x2.astype(ml_dtypes.bfloat16)
    wf = (w2_1 @ w1)[0]
    f = wf @ x2
    fmax = float(np.abs(f).max()) * 1.05 + 0.05
    ep = np.array([[-fmax, 2.0 * fmax / NE, fmax, -2.0 * fmax / NE]],
                  dtype=np.float32)
    in_maps = []
    for c in range(CORES):
        in_maps.append({
            "x": xbf,
            "xI": np.ascontiguousarray(xbf[:, c * IC:(c + 1) * IC]),
            "w1": w1,
            "w1T": np.ascontiguousarray(w1.T),
            "w2T": np.ascontiguousarray(w2_1.T),
            "ep": ep,
        })
    res = run_bass_kernel_spmd(nc, in_maps, core_ids=list(range(CORES)))
    y = np.concatenate([res.results[c]["y"] for c in range(CORES)], axis=1)
    return y[None].astype(np.float32)


if __name__ == "__main__":
    rng = np.random.default_rng(0)
    N = 8192
    x = rng.standard_normal((1, CIN, N), dtype=np.float32)
    w1 = (rng.standard_normal((COUT, CIN)) / np.sqrt(CIN)).astype(np.float32)
    w2 = (rng.standard_normal((1, COUT)) / np.sqrt(COUT)).astype(np.float32)
    bias = np.zeros((N, N), np.float32)
    y = kernel(x=x, bias_mat=bias, w1=w1, w2_1=w2)
    print("kernel output", y.shape, y.dtype)
